# revision 11
# baseline (speedup 1.0000x reference)
"""2-layer LSTM (SEQ=512, B=64, H=1024, IN=1024) on 8 Trainium2 NeuronCores.

Strategy: tensor-parallel striped split of the 4H gate dimension across the 8
cores (core c owns h rows [128c, 128c+128) of every gate), with one fused
AllGather of both layers' h.T chunks per time step.  Gates are computed in
transposed layout (gate rows on partitions, batch on the free dim) so the
recurrence needs no per-step transposes.  Layer 1 lags layer 0 by LAG steps;
its input projection is computed on-device from the gathered h0.T stream.
All matmuls run in bf16 with fp32 PSUM accumulation.
"""

import os
import numpy as np
import ml_dtypes

import concourse.bass as bass
import concourse.bacc as bacc
import concourse.mybir as mybir
import concourse.tile as tile
from concourse.bass_utils import run_bass_kernel_spmd

BF16 = ml_dtypes.bfloat16

T, B, H, IN, C = 512, 64, 1024, 1024, 8
G = 4 * H          # 4096 gate rows
MCH = 4            # gate m-tiles per core (each [128, *])
KT = IN // 128     # 8 contraction k-tiles
CHUNK = 16         # steps per projection chunk
LAG = 24           # layer-1 step lag behind layer-0

_BUILD_CACHE = {}


def _build(t_steps):
    total = t_steps + LAG
    n_chunks = t_steps // CHUNK
    steps_per_core = t_steps // C

    nc = bacc.Bacc(num_devices=C)
    f32, bf16 = mybir.dt.float32, mybir.dt.bfloat16

    # ---- external I/O (per-core) ----
    x_my = nc.dram_tensor("x_my", [steps_per_core * B, IN], f32, kind="ExternalInput")
    w0iT = nc.dram_tensor("w0iT", [IN, 512], bf16, kind="ExternalInput")
    w0hT = nc.dram_tensor("w0hT", [H, 512], bf16, kind="ExternalInput")
    w1iT = nc.dram_tensor("w1iT", [H, 512], bf16, kind="ExternalInput")
    w1hT = nc.dram_tensor("w1hT", [H, 512], bf16, kind="ExternalInput")
    bias0 = nc.dram_tensor("bias0", [128, MCH], f32, kind="ExternalInput")
    bias1 = nc.dram_tensor("bias1", [128, MCH], f32, kind="ExternalInput")
    ident_bf = nc.dram_tensor("ident_bf", [128, 128], bf16, kind="ExternalInput")
    ident_f32 = nc.dram_tensor("ident_f32", [128, 128], f32, kind="ExternalInput")

    out1_my = nc.dram_tensor("out1_my", [steps_per_core, B, H], f32,
                             kind="ExternalOutput")
    hn_my = nc.dram_tensor("hn_my", [2, B, 128], f32, kind="ExternalOutput")
    cn_my = nc.dram_tensor("cn_my", [2, B, 128], f32, kind="ExternalOutput")

    AF = mybir.ActivationFunctionType
    rg = [list(range(C))]

    with tile.TileContext(nc) as tc:
        with (
            tc.tile_pool(name="wpool", bufs=1) as wpool,
            tc.tile_pool(name="xppool", bufs=1) as xppool,
            tc.tile_pool(name="stage", bufs=10) as stagepool,
            tc.tile_pool(name="loop", bufs=3) as looppool,
            tc.tile_pool(name="state", bufs=1) as statepool,
            tc.tile_pool(name="psum", bufs=2, space="PSUM") as psumpool,
            tc.tile_pool(name="dram", bufs=1, space="DRAM") as dram,
            tc.tile_pool(name="dramdin", bufs=3, space="DRAM") as dramdin,
        ):
            # ---------- load weights / constants into SBUF ----------
            wt = {}
            for name, src in (("w0i", w0iT), ("w0h", w0hT),
                              ("w1i", w1iT), ("w1h", w1hT)):
                t_ = wpool.tile([128, KT * 512], bf16, tag=f"w_{name}")
                # tile k holds WT rows [128k,128k+128) -> cols [512k, 512k+512)
                nc.sync.dma_start(
                    t_[:].rearrange("p (k m) -> p k m", k=KT),
                    src[:].rearrange("(k p) m -> p k m", p=128))
                wt[name] = t_
            ident = wpool.tile([128, 128], bf16, tag="ident")
            nc.sync.dma_start(ident[:], ident_bf[:])
            identf = wpool.tile([128, 128], f32, tag="identf")
            nc.sync.dma_start(identf[:], ident_f32[:])
            bias_sb = {}
            for l, src in ((0, bias0), (1, bias1)):
                bt = wpool.tile([128, MCH], f32, tag=f"bias{l}")
                nc.sync.dma_start(bt[:], src[:])
                bias_sb[l] = bt

            # ---------- phase 0: transpose my x slice to xT (bf16) ----------
            xT_mine = dram.tile([IN, steps_per_core * B], bf16)
            n_tok_tiles = (steps_per_core * B) // 128
            with tc.tile_pool(name="ph0", bufs=2) as ph0:
                for tt in range(n_tok_tiles):
                    xin = ph0.tile([128, IN], f32, tag="xin")
                    nc.sync.dma_start(xin[:], x_my[tt * 128:(tt + 1) * 128, :])
                    xbf = ph0.tile([128, IN], bf16, tag="xbf")
                    nc.vector.tensor_copy(xbf[:], xin[:])
                    xtr = ph0.tile([128, IN], bf16, tag="xtr")
                    for k in range(KT):
                        nc.sync.dma_start(
                            xtr[:, 128 * k:128 * (k + 1)],
                            xbf[:, 128 * k:128 * (k + 1)], transpose=True)
                    # write all k tiles: row 128k+p, col tt*128..+128
                    nc.sync.dma_start(
                        xT_mine[:, tt * 128:(tt + 1) * 128]
                        .rearrange("(k p) c -> p k c", p=128),
                        xtr[:].rearrange("p (k c) -> p k c", k=KT))

            # share xT across cores (also acts as the phase barrier)
            xT_full = dram.tile([C, IN, steps_per_core * B], bf16, addr_space="Shared")
            nc.gpsimd.collective_compute(
                "AllGather", mybir.AluOpType.bypass,
                ins=[xT_mine[:].opt()], outs=[xT_full[:].opt()],
                replica_groups=rg)

            # gathered h (both layers) per step, plus din staging
            hT_all = dram.tile([total, H, 128], bf16)

            # persistent cell state + final h capture
            cst = [statepool.tile([128, B], f32, tag=f"c{l}", name=f"cst{l}")
                   for l in range(2)]
            for l in range(2):
                nc.vector.memset(cst[l][:], 0.0)
            hfin = [statepool.tile([128, B], f32, tag=f"hf{l}", name=f"hfin{l}")
                    for l in range(2)]

            def proj_chunk(l, q, wname, bias_t):
                """Project chunk q of layer l input into xp tile (bf16)."""
                xpc = xppool.tile([128, MCH * CHUNK * B], bf16,
                                  tag=f"xp{l}_{q % 3}")
                w = wt[wname]
                ncols = CHUNK * B  # 1024
                stages = []
                for k in range(KT):
                    st = stagepool.tile([128, ncols], bf16, tag=f"projstage{l}", name="st",
                                        bufs=18 if l == 0 else 10)
                    if l == 0:
                        r, lo = q // (steps_per_core // CHUNK), q % (steps_per_core // CHUNK)
                        nc.sync.dma_start(
                            st[:], xT_full[r, 128 * k:128 * (k + 1),
                                           lo * ncols:(lo + 1) * ncols])
                    else:
                        # strided read of hT_all[16q+s][128k+p, 0:64]
                        src = hT_all[CHUNK * q:CHUNK * (q + 1),
                                     128 * k:128 * (k + 1), 0:64]
                        nc.sync.dma_start(
                            st[:].rearrange("p (s b) -> p s b", s=CHUNK),
                            src.rearrange("s p b -> p s b"))
                    stages.append(st)
                for m in range(MCH):
                    for half in range(2):
                        ps = psumpool.tile([128, 512], f32, tag="projpsum")
                        for k in range(KT):
                            st = stages[k]
                            nc.tensor.matmul(
                                ps[:],
                                w[:, 512 * k + 128 * m:512 * k + 128 * (m + 1)],
                                st[:, 512 * half:512 * (half + 1)],
                                start=(k == 0), stop=(k == KT - 1))
                        nc.scalar.activation(
                            xpc[:, m * 1024 + 512 * half:m * 1024 + 512 * (half + 1)],
                            ps[:], AF.Identity, bias=bias_t[:, m:m + 1])
                return xpc

            # layer-0 projections: trace 2 chunks ahead of consumption so
            # scheduler priority matches execution order (no slot inversion)
            xp0 = {}
            for q in range(min(2, n_chunks)):
                xp0[q] = proj_chunk(0, q, "w0i", bias_sb[0])
            xp1 = {}

            for t in range(total):
                if t % CHUNK == 0:
                    q = t // CHUNK + 2
                    if q < n_chunks:
                        xp0[q] = proj_chunk(0, q, "w0i", bias_sb[0])
                # per-step landing of previous gather
                if t > 0:
                    land = looppool.tile([128, H], bf16, tag="land")
                    nc.sync.dma_start(
                        land[:].rearrange("p (j c) -> p j c", j=KT),
                        hT_all[t - 1].rearrange("(j p) c -> p j c", p=128))
                else:
                    land = None

                hch = looppool.tile([128, 128], bf16, tag="hch")
                for l in range(2):
                    u = t - LAG * l
                    if not (0 <= u < t_steps):
                        nc.vector.memset(hch[:, 64 * l:64 * (l + 1)], 0.0)
                        continue
                    wname = "w0h" if l == 0 else "w1h"
                    w = wt[wname]
                    xpc = xp0[u // CHUNK] if l == 0 else xp1[u // CHUNK]
                    g = psumpool.tile([128, 256], f32, tag=f"gpsum{l}")
                    for m in range(MCH):
                        if u > 0:
                            for k in range(KT):
                                nc.tensor.matmul(
                                    g[:, 64 * m:64 * (m + 1)],
                                    w[:, 512 * k + 128 * m:512 * k + 128 * (m + 1)],
                                    land[:, 128 * k + 64 * l:128 * k + 64 * l + 64],
                                    start=(k == 0), stop=False)
                        nc.tensor.matmul(
                            g[:, 64 * m:64 * (m + 1)], ident[:],
                            xpc[:, m * 1024 + (u % CHUNK) * B:
                                m * 1024 + (u % CHUNK) * B + B],
                            start=(u == 0), stop=True)
                    # gates: m=0:i 1:f 2:g 3:o
                    sig_if = looppool.tile([128, 128], f32, tag=f"sif{l}")
                    nc.scalar.activation(sig_if[:], g[:, 0:128], AF.Sigmoid)
                    tg = looppool.tile([128, B], f32, tag=f"tg{l}")
                    nc.scalar.activation(tg[:], g[:, 128:192], AF.Tanh)
                    sig_o = looppool.tile([128, B], f32, tag=f"so{l}")
                    nc.scalar.activation(sig_o[:], g[:, 192:256], AF.Sigmoid)

                    t1 = looppool.tile([128, B], f32, tag=f"t1{l}")
                    nc.vector.tensor_mul(t1[:], sig_if[:, 0:64], tg[:])
                    t2 = looppool.tile([128, B], f32, tag=f"t2{l}")
                    nc.vector.tensor_mul(t2[:], sig_if[:, 64:128], cst[l][:])
                    nc.vector.tensor_add(cst[l][:], t1[:], t2[:])
                    tc_ = looppool.tile([128, B], f32, tag=f"tc{l}")
                    nc.scalar.activation(tc_[:], cst[l][:], AF.Tanh)
                    nc.vector.tensor_mul(hch[:, 64 * l:64 * (l + 1)],
                                         sig_o[:], tc_[:])
                    if u == t_steps - 1:
                        nc.vector.tensor_mul(hfin[l][:], sig_o[:], tc_[:])

                din = dramdin.tile([128, 128], bf16, tag="din")
                nc.sync.dma_start(din[:], hch[:])
                nc.gpsimd.collective_compute(
                    "AllGather", mybir.AluOpType.bypass,
                    ins=[din[:].opt()], outs=[hT_all[t][:].opt()],
                    replica_groups=rg)

                # trace layer-1 projection for chunk q right after its last AG
                if (t + 1) % CHUNK == 0:
                    q = (t + 1) // CHUNK - 1
                    if q < n_chunks:
                        xp1[q] = proj_chunk(1, q, "w1i", bias_sb[1])

            # ---------- final: write out1 (my steps), h_n, c_n ----------
            pid = nc.sync.partition_id()
            with tc.tile_pool(name="fin", bufs=2) as fin, \
                 tc.tile_pool(name="finps", bufs=2, space="PSUM") as finps:
                for i in range(steps_per_core):
                    hb = fin.tile([128, 512], bf16, tag="hb")
                    src = hT_all[bass.ds(pid * steps_per_core + (i + LAG), 1),
                                 :, 64:128]
                    nc.sync.dma_start(
                        hb[:].rearrange("p (j b) -> p j b", j=KT),
                        src.rearrange("one (j p) b -> p (one j) b", p=128))
                    ptr = finps.tile([64, KT * 128], bf16, tag="ptr")
                    for j in range(KT):
                        nc.tensor.transpose(ptr[:, 128 * j:128 * (j + 1)],
                                            hb[:, 64 * j:64 * (j + 1)], ident[:])
                    ob = fin.tile([64, H], f32, tag="ob")
                    nc.vector.tensor_copy(ob[:], ptr[:])
                    nc.sync.dma_start(out1_my[i][:], ob[:])

                for l in range(2):
                    for name, st, dst in (("h", hfin[l], hn_my),
                                          ("c", cst[l], cn_my)):
                        pt = finps.tile([64, 128], f32, tag="ptr")
                        nc.tensor.transpose(pt[:], st[:], identf[:])
                        sb = fin.tile([64, 128], f32, tag="hcsb")
                        nc.vector.tensor_copy(sb[:], pt[:])
                        nc.sync.dma_start(dst[l][:], sb[:])

    nc.compile()
    return nc


def _shard_inputs(inputs, t_steps):
    x = np.asarray(inputs["x"], np.float32)[:t_steps]
    steps_per_core = t_steps // C
    ident_bf = np.eye(128, dtype=BF16)
    ident_f32 = np.eye(128, dtype=np.float32)

    def wchunk(wname, c):
        w = np.asarray(inputs[wname], np.float32)
        # striped rows: gate m rows [1024m + 128c, +128)
        rows = np.concatenate(
            [w[1024 * m + 128 * c:1024 * m + 128 * (c + 1)] for m in range(MCH)],
            axis=0)  # [512, K]
        return np.ascontiguousarray(rows.T.astype(BF16))  # [K, 512]

    def bchunk(l, c):
        b = (np.asarray(inputs[f"b_ih{l}"], np.float32)
             + np.asarray(inputs[f"b_hh{l}"], np.float32))
        cols = np.stack(
            [b[1024 * m + 128 * c:1024 * m + 128 * (c + 1)] for m in range(MCH)],
            axis=1)  # [128, 4]
        return np.ascontiguousarray(cols)

    in_maps = []
    for c in range(C):
        xs = x[c * steps_per_core:(c + 1) * steps_per_core]  # [spc, B, IN]
        in_maps.append({
            "x_my": np.ascontiguousarray(
                xs.reshape(steps_per_core * B, IN)).astype(np.float32),
            "w0iT": wchunk("W_ih0", c),
            "w0hT": wchunk("W_hh0", c),
            "w1iT": wchunk("W_ih1", c),
            "w1hT": wchunk("W_hh1", c),
            "bias0": bchunk(0, c),
            "bias1": bchunk(1, c),
            "ident_bf": ident_bf,
            "ident_f32": ident_f32,
        })
    return in_maps


def run_kernel(inputs, t_steps=T):
    if t_steps not in _BUILD_CACHE:
        _BUILD_CACHE[t_steps] = _build(t_steps)
    nc = _BUILD_CACHE[t_steps]
    in_maps = _shard_inputs(inputs, t_steps)
    res = run_bass_kernel_spmd(nc, in_maps, core_ids=list(range(C)))
    steps_per_core = t_steps // C
    out1 = np.zeros((t_steps, B, H), np.float32)
    h_n = np.zeros((2, B, H), np.float32)
    c_n = np.zeros((2, B, H), np.float32)
    for c in range(C):
        r = res.results[c]
        out1[c * steps_per_core:(c + 1) * steps_per_core] = r["out1_my"]
        h_n[:, :, 128 * c:128 * (c + 1)] = r["hn_my"]
        c_n[:, :, 128 * c:128 * (c + 1)] = r["cn_my"]
    return out1, h_n, c_n


def kernel(**inputs):
    return run_kernel(inputs, T)
